# revision 14
# baseline (speedup 1.0000x reference)
"""Trainium2 Bass kernel for a hard-triplet margin-ranking loss.

Sharding: data-parallel over anchor rows. 8 cores x 512 rows each. Rows in
the first half of the batch mine over columns [2048:4096], rows in the second
half over [0:2048]; each core computes its 512x2048 slice of the distance
matrix. Per core:

  1. DMA: targets (tiny) first, then anchors xb, then 4 opposite groups.
  2. Row norms: squares split between ACT (Square+accum) and DVE
     (tensor_tensor_reduce mult/add); inv = 1/(sqrt(sq)+eps).
  3. Normalization is folded into the PE transpose: instead of an identity
     rhs, the transpose uses diag(inv) (diag(-0.25*inv) for anchors), so the
     transposed tiles come out pre-scaled and pm = -0.25*<xn_i, xn_j>.
  4. Main matmuls run as float32r (bit-identical to f32, 1 cycle/row for
     512-col outputs vs 4 for plain f32) into [128,1024] PSUM halves.
  5. Same-class mask M[r] = (t_o == t_b_r) precomputed as fp16 via 2x-mode
     tensor_scalar is_equal during the DMA phase.  Row max via ONE fused
     tensor_tensor_reduce per half: w = M + pm (fp16, SBUF) + max accum.
  6. Row min via 2x-mode TT min tree on the fp16 w tiles.
  7. dist_ap = sqrt(relu(8*mx - 6)), dist_an from 8*mn + 2 as in the
     reference epilogue; ones-matmul row-sum; host sums 8 partials / 4096.
"""

import numpy as np

N, D = 4096, 256
HALF = N // 2
NCORES = 8
RPC = N // NCORES  # 512 anchor rows per core
RT = RPC // 128    # 4 anchor row tiles
NG = 5             # group 0 = anchors, 1..4 = opposite half
MARGIN = 0.3
EPS = 1e-6

_CACHE = {}


def _build():
    from contextlib import ExitStack

    import concourse.bacc as bacc
    import concourse.bass as bass
    import concourse.tile as tile
    from concourse import masks, mybir

    f32 = mybir.dt.float32
    f32r = mybir.dt.float32r
    fp16 = mybir.dt.float16
    Alu = mybir.AluOpType
    Act = mybir.ActivationFunctionType
    AxX = mybir.AxisListType.X
    ts = bass.ts

    nc = bacc.Bacc(
        "TRN2",
        target_bir_lowering=False,
        debug=False,
        enable_asserts=True,
        num_devices=NCORES,
    )
    xb = nc.dram_tensor("xb", [128, RT * D], f32, kind="ExternalInput").ap()
    xo = nc.dram_tensor("xo", [128, 16 * D], f32, kind="ExternalInput").ap()
    tb = nc.dram_tensor("tb", [128, RT], f32, kind="ExternalInput").ap()
    to = nc.dram_tensor("to", [1, HALF], fp16, kind="ExternalInput").ap()
    out = nc.dram_tensor("out", [1, 1], f32, kind="ExternalOutput").ap()

    with tile.TileContext(nc) as tc, ExitStack() as ctx:
        const = ctx.enter_context(tc.tile_pool(name="const", bufs=1))
        xin = ctx.enter_context(tc.tile_pool(name="xin", bufs=1))
        xt = ctx.enter_context(tc.tile_pool(name="xt", bufs=1))
        stat = ctx.enter_context(tc.tile_pool(name="stat", bufs=1))
        scr = ctx.enter_context(tc.tile_pool(name="scr", bufs=4))
        dpool = ctx.enter_context(tc.tile_pool(name="dpool", bufs=4))
        wide = ctx.enter_context(tc.tile_pool(name="wide", bufs=1))
        tp = ctx.enter_context(tc.tile_pool(name="tp", bufs=1, space="PSUM"))
        pmp = ctx.enter_context(tc.tile_pool(name="pmp", bufs=3, space="PSUM"))

        # --- DMAs: tiny target tensors first, then features.
        to_row = const.tile([1, HALF], fp16, tag="to_row")
        nc.sync.dma_start(to_row[:], to[:])
        tbt = const.tile([128, RT], f32, tag="tbt")
        nc.sync.dma_start(tbt[:], tb[:])
        xg = []
        for g in range(NG):
            gt = xin.tile([128, 1024], f32, tag=f"xg{g}")
            if g == 0:
                nc.sync.dma_start(gt[:], xb[:])
            else:
                nc.sync.dma_start(gt[:], xo[:, (g - 1) * 1024 : g * 1024])
            xg.append(gt)

        # --- t~0 constants; dummy Sqrt first so the single act table set
        # loaded is sqrt_and_others (contains Square, Sqrt, Copy).
        sc0 = const.tile([128, 1], f32, tag="sc0")
        nc.vector.memset(sc0[:], 1.0)
        sc1 = const.tile([128, 1], f32, tag="sc1")
        nc.scalar.activation(sc1[:], sc0[:], Act.Sqrt)
        ident = const.tile([128, 128], f32, tag="ident")
        masks.make_identity(nc, ident[:])
        ones = const.tile([128, 1], f32, tag="ones")
        nc.vector.memset(ones[:], 1.0)
        tob = const.tile([128, HALF], fp16, tag="tob")
        nc.gpsimd.partition_broadcast(tob[:], to_row[:])

        for _ in range(8):
            wp = tp.tile([128, 128], f32, tag="tp", name="wp")
            nc.tensor.matmul(wp[:], lhsT=ident[:], rhs=ident[:],
                             start=True, stop=True)

        sq = stat.tile([128, 4 * NG], f32, tag="sq")
        inv = stat.tile([128, 4 * NG], f32, tag="inv")

        xT = []
        wtiles = []
        for r in range(RT):
            wtiles.append(wide.tile([128, HALF], fp16, tag=f"w{r}", name=f"w{r}"))

        def group_norms(g):
            # squares: tiles 4g..4g+3; 2 on ACT, 2 on DVE.
            for i in range(4):
                t = 4 * g + i
                xs = scr.tile([128, D], f32, tag="xs")
                if i < 2:
                    nc.scalar.activation(
                        xs[:], xg[g][:, ts(i, D)], Act.Square,
                        accum_out=sq[:, t : t + 1],
                    )
                else:
                    nc.vector.scalar_tensor_tensor(
                        xs[:], xg[g][:, ts(i, D)], 1.0, xg[g][:, ts(i, D)],
                        op0=Alu.mult, op1=Alu.mult, accum_out=sq[:, t : t + 1],
                    )
            s4 = slice(4 * g, 4 * g + 4)
            nrm = scr.tile([128, 4], f32, tag="nrm")
            nc.scalar.activation(nrm[:], sq[:, s4], Act.Sqrt)
            nrme = scr.tile([128, 4], f32, tag="nrme")
            nc.vector.tensor_scalar_add(nrme[:], nrm[:], EPS)
            if g == 0:
                # anchors carry the -0.25 prescale
                iv = scr.tile([128, 4], f32, tag="iv")
                nc.vector.reciprocal(iv[:], nrme[:])
                nc.vector.tensor_scalar_mul(inv[:, s4], iv[:], -0.25)
            else:
                nc.vector.reciprocal(inv[:, s4], nrme[:])

        def group_transpose(g, evac):
            gt = xt.tile([128, 1024], f32r, tag=f"xT{g}")
            pt = tp.tile([128, 1024], f32, tag="tp")
            for i in range(4):
                t = 4 * g + i
                xn = dpool.tile([128, D], f32, tag="xn")
                if g > 0 and i >= 2:
                    nc.gpsimd.tensor_scalar_mul(
                        xn[:], xg[g][:, ts(i, D)], inv[:, t : t + 1]
                    )
                else:
                    nc.vector.tensor_scalar_mul(
                        xn[:], xg[g][:, ts(i, D)], inv[:, t : t + 1]
                    )
                for c in range(2):
                    nc.tensor.transpose(
                        pt[:, ts(c * 4 + i, 128)], xn[:, ts(c, 128)], ident[:]
                    )
            if evac == "act":
                nc.scalar.copy(gt[:], pt[:])
            elif evac == "dve":
                nc.vector.tensor_copy(gt[:], pt[:])
            else:  # split act + dve
                nc.scalar.copy(gt[:, 0:512], pt[:, 0:512])
                nc.vector.tensor_copy(gt[:, 512:1024], pt[:, 512:1024])
            xT.append(gt)

        # Groups 0..2: norms, transposes, masks.
        evac_plan = ["act", "act", "split", "act", "split"]
        for g in range(3):
            group_norms(g)
            group_transpose(g, evac_plan[g])
        for g in range(3, NG):
            group_norms(g)
            group_transpose(g, evac_plan[g])

        # --- Matmuls in halves: A = groups 1,2 (cols 0:1024), B = 3,4.
        lhs = xT[0]
        mx = stat.tile([128, RT], f32, tag="mx")
        mn = stat.tile([128, RT], f32, tag="mn")

        def half_mms(r, gi0):
            pm = pmp.tile([128, 1024], f32, tag="pm")
            for k, gi in enumerate((gi0, gi0 + 1)):
                for c in range(2):
                    nc.tensor.matmul(
                        pm[:, ts(k, 512)],
                        lhsT=lhs[:, c * 512 + r * 128 : c * 512 + (r + 1) * 128],
                        rhs=xT[gi][:, ts(c, 512)],
                        start=(c == 0),
                        stop=(c == 1),
                    )
            return pm

        pma = [half_mms(r, 1) for r in range(RT)]

        def stt(r, pm, h):
            # w = (t_o == t_b_r) + pm, fused mask with the PSUM read.
            nc.vector.scalar_tensor_tensor(
                wtiles[r][:, h * 1024 : (h + 1) * 1024],
                tob[:, h * 1024 : (h + 1) * 1024],
                tbt[:, r : r + 1],
                pm[:],
                op0=Alu.is_equal,
                op1=Alu.add,
            )

        def tree(r, op, acc):
            w = wtiles[r]
            t1 = scr.tile([128, 1024], fp16, tag="t1")
            nc.vector.tensor_tensor(t1[:], w[:, 0:1024], w[:, 1024:2048], op=op)
            t2 = scr.tile([128, 512], fp16, tag="t2")
            nc.vector.tensor_tensor(t2[:], t1[:, 0:512], t1[:, 512:1024], op=op)
            t3 = scr.tile([128, 256], fp16, tag="t3")
            nc.vector.tensor_tensor(t3[:], t2[:, 0:256], t2[:, 256:512], op=op)
            nc.vector.tensor_reduce(acc[:, r : r + 1], t3[:], axis=AxX, op=op)

        for r in range(3):
            stt(r, pma[r], 0)
        pmb = [half_mms(r, 3) for r in range(RT)]
        stt(3, pma[3], 0)
        for r in range(RT):
            stt(r, pmb[r], 1)
            tree(r, Alu.max, mx)
            tree(r, Alu.min, mn)

        # --- Epilogue on [128, RT].
        u1 = stat.tile([128, RT], f32, tag="u1")
        nc.vector.tensor_scalar(u1[:], mx[:], 8.0, -6.0, op0=Alu.mult, op1=Alu.add)
        u = stat.tile([128, RT], f32, tag="u")
        nc.vector.tensor_scalar_max(u[:], u1[:], 0.0)
        dap = stat.tile([128, RT], f32, tag="dap")
        nc.scalar.activation(dap[:], u[:], Act.Sqrt)
        v1 = stat.tile([128, RT], f32, tag="v1")
        nc.vector.tensor_scalar(v1[:], mn[:], 8.0, 2.0, op0=Alu.mult, op1=Alu.add)
        v = stat.tile([128, RT], f32, tag="v")
        nc.vector.tensor_scalar_max(v[:], v1[:], EPS)
        sv = stat.tile([128, RT], f32, tag="sv")
        nc.scalar.activation(sv[:], v[:], Act.Sqrt)
        e = stat.tile([128, RT], f32, tag="e")
        nc.vector.tensor_scalar(e[:], v[:], 6.0, None, op0=Alu.is_gt)
        ome = stat.tile([128, RT], f32, tag="ome")
        nc.vector.tensor_scalar(ome[:], e[:], -1.0, 1.0, op0=Alu.mult, op1=Alu.add)
        t1 = stat.tile([128, RT], f32, tag="t1e")
        nc.vector.tensor_tensor(t1[:], sv[:], ome[:], op=Alu.mult)
        dan = stat.tile([128, RT], f32, tag="dan")
        nc.vector.tensor_tensor(dan[:], t1[:], e[:], op=Alu.add)
        df = stat.tile([128, RT], f32, tag="df")
        nc.vector.tensor_tensor(df[:], dap[:], dan[:], op=Alu.subtract)
        lrow = stat.tile([128, RT], f32, tag="lrow")
        nc.vector.tensor_scalar(
            lrow[:], df[:], MARGIN, 0.0, op0=Alu.add, op1=Alu.max
        )
        ps2 = tp.tile([1, RT], f32, tag="tp")
        nc.tensor.matmul(ps2[:], lhsT=ones[:], rhs=lrow[:], start=True, stop=True)
        tot = stat.tile([1, 1], f32, tag="tot")
        nc.vector.tensor_reduce(tot[:], ps2[:], axis=AxX, op=Alu.add)
        nc.sync.dma_start(out[:], tot[:])

    nc.compile()
    return nc


def _get_nc():
    if "nc" not in _CACHE:
        _CACHE["nc"] = _build()
    return _CACHE["nc"]


def make_in_maps(inputs: np.ndarray, targets: np.ndarray):
    inputs = np.ascontiguousarray(inputs, dtype=np.float32)
    tf = targets.astype(np.float32)
    th = targets.astype(np.float16)
    in_maps = []
    for r in range(NCORES):
        rows = slice(r * RPC, (r + 1) * RPC)
        opp = slice(HALF, N) if r * RPC < HALF else slice(0, HALF)
        in_maps.append(
            {
                # partition p holds rows 4p..4p+3 (contiguous 4KB DMA);
                # "tile" t within a group is row 4p+t.
                "xb": inputs[rows].reshape(128, RT * D),
                "xo": inputs[opp].reshape(128, 16 * D),
                "tb": tf[rows].reshape(128, RT),
                # xo partition k holds rows 16k..16k+15; group n covers tile
                # slices 4n..4n+3, so distance column n*512 + i*128 + k is
                # xo-row 16k + 4n + i: permute targets to match.
                "to": th[opp].reshape(128, 4, 4).transpose(1, 2, 0).reshape(1, HALF),
            }
        )
    return in_maps


def kernel(inputs: np.ndarray, targets: np.ndarray) -> np.ndarray:
    from concourse.bass_utils import run_bass_kernel_spmd

    nc = _get_nc()
    in_maps = make_in_maps(inputs, targets)
    res = run_bass_kernel_spmd(nc, in_maps, list(range(NCORES)))
    total = sum(float(res.results[i]["out"][0, 0]) for i in range(NCORES))
    return np.float32(total / N)


# revision 15
# speedup vs baseline: 1.0195x; 1.0195x over previous
"""Trainium2 Bass kernel for a hard-triplet margin-ranking loss.

Sharding: data-parallel over anchor rows. 8 cores x 512 rows each. Rows in
the first half of the batch mine over columns [2048:4096], rows in the second
half over [0:2048]; each core computes its 512x2048 slice of the distance
matrix. Per core:

  1. DMA: targets (tiny) first, then anchors xb, then 4 opposite groups.
  2. Row norms: squares split between ACT (Square+accum) and DVE
     (tensor_tensor_reduce mult/add); inv = 1/(sqrt(sq)+eps).
  3. Normalization is folded into the PE transpose: instead of an identity
     rhs, the transpose uses diag(inv) (diag(-0.25*inv) for anchors), so the
     transposed tiles come out pre-scaled and pm = -0.25*<xn_i, xn_j>.
  4. Main matmuls run as float32r (bit-identical to f32, 1 cycle/row for
     512-col outputs vs 4 for plain f32) into [128,1024] PSUM halves.
  5. Same-class mask M[r] = (t_o == t_b_r) precomputed as fp16 via 2x-mode
     tensor_scalar is_equal during the DMA phase.  Row max via ONE fused
     tensor_tensor_reduce per half: w = M + pm (fp16, SBUF) + max accum.
  6. Row min via 2x-mode TT min tree on the fp16 w tiles.
  7. dist_ap = sqrt(relu(8*mx - 6)), dist_an from 8*mn + 2 as in the
     reference epilogue; ones-matmul row-sum; host sums 8 partials / 4096.
"""

import numpy as np

N, D = 4096, 256
HALF = N // 2
NCORES = 8
RPC = N // NCORES  # 512 anchor rows per core
RT = RPC // 128    # 4 anchor row tiles
NG = 5             # group 0 = anchors, 1..4 = opposite half
MARGIN = 0.3
EPS = 1e-6

_CACHE = {}


def _build():
    from contextlib import ExitStack

    import concourse.bacc as bacc
    import concourse.bass as bass
    import concourse.tile as tile
    from concourse import masks, mybir

    f32 = mybir.dt.float32
    f32r = mybir.dt.float32r
    fp16 = mybir.dt.float16
    Alu = mybir.AluOpType
    Act = mybir.ActivationFunctionType
    AxX = mybir.AxisListType.X
    ts = bass.ts

    nc = bacc.Bacc(
        "TRN2",
        target_bir_lowering=False,
        debug=False,
        enable_asserts=True,
        num_devices=NCORES,
    )
    xb = nc.dram_tensor("xb", [128, RT * D], f32, kind="ExternalInput").ap()
    xo = nc.dram_tensor("xo", [128, 16 * D], f32, kind="ExternalInput").ap()
    tb = nc.dram_tensor("tb", [128, RT], f32, kind="ExternalInput").ap()
    to = nc.dram_tensor("to", [1, HALF], fp16, kind="ExternalInput").ap()
    out = nc.dram_tensor("out", [1, 1], f32, kind="ExternalOutput").ap()

    with tile.TileContext(nc) as tc, ExitStack() as ctx:
        const = ctx.enter_context(tc.tile_pool(name="const", bufs=1))
        xin = ctx.enter_context(tc.tile_pool(name="xin", bufs=1))
        xt = ctx.enter_context(tc.tile_pool(name="xt", bufs=1))
        stat = ctx.enter_context(tc.tile_pool(name="stat", bufs=1))
        scr = ctx.enter_context(tc.tile_pool(name="scr", bufs=4))
        dpool = ctx.enter_context(tc.tile_pool(name="dpool", bufs=4))
        wide = ctx.enter_context(tc.tile_pool(name="wide", bufs=1))
        tp = ctx.enter_context(tc.tile_pool(name="tp", bufs=1, space="PSUM"))
        pmp = ctx.enter_context(tc.tile_pool(name="pmp", bufs=3, space="PSUM"))

        # --- DMAs: tiny target tensors first, then features.
        to_row = const.tile([1, HALF], fp16, tag="to_row")
        nc.sync.dma_start(to_row[:], to[:])
        tbt = const.tile([128, RT], f32, tag="tbt")
        nc.sync.dma_start(tbt[:], tb[:])
        xg = []
        for g in range(NG):
            gt = xin.tile([128, 1024], f32, tag=f"xg{g}")
            if g == 0:
                nc.sync.dma_start(gt[:], xb[:])
            else:
                nc.sync.dma_start(gt[:], xo[:, (g - 1) * 1024 : g * 1024])
            xg.append(gt)

        # --- t~0 constants; dummy Sqrt first so the single act table set
        # loaded is sqrt_and_others (contains Square, Sqrt, Copy).
        sc0 = const.tile([128, 1], f32, tag="sc0")
        nc.vector.memset(sc0[:], 1.0)
        sc1 = const.tile([128, 1], f32, tag="sc1")
        nc.scalar.activation(sc1[:], sc0[:], Act.Sqrt)
        ident = const.tile([128, 128], f32, tag="ident")
        masks.make_identity(nc, ident[:])
        ones = const.tile([128, 1], f32, tag="ones")
        nc.vector.memset(ones[:], 1.0)
        tob = const.tile([128, HALF], fp16, tag="tob")
        nc.gpsimd.partition_broadcast(tob[:], to_row[:])

        for _ in range(8):
            wp = tp.tile([128, 128], f32, tag="tp", name="wp")
            nc.tensor.matmul(wp[:], lhsT=ident[:], rhs=ident[:],
                             start=True, stop=True)

        sq = stat.tile([128, 4 * NG], f32, tag="sq")
        inv = stat.tile([128, 4 * NG], f32, tag="inv")

        xT = []
        wtiles = []
        for r in range(RT):
            wtiles.append(wide.tile([128, HALF], fp16, tag=f"w{r}", name=f"w{r}"))

        def group_norms(g):
            # squares: tiles 4g..4g+3; 2 on ACT, 2 on DVE.
            for i in range(4):
                t = 4 * g + i
                xs = scr.tile([128, D], f32, tag="xs")
                if i < 2:
                    nc.scalar.activation(
                        xs[:], xg[g][:, ts(i, D)], Act.Square,
                        accum_out=sq[:, t : t + 1],
                    )
                else:
                    nc.vector.scalar_tensor_tensor(
                        xs[:], xg[g][:, ts(i, D)], 1.0, xg[g][:, ts(i, D)],
                        op0=Alu.mult, op1=Alu.mult, accum_out=sq[:, t : t + 1],
                    )
            s4 = slice(4 * g, 4 * g + 4)
            nrm = scr.tile([128, 4], f32, tag="nrm")
            nc.scalar.activation(nrm[:], sq[:, s4], Act.Sqrt)
            nrme = scr.tile([128, 4], f32, tag="nrme")
            nc.vector.tensor_scalar_add(nrme[:], nrm[:], EPS)
            if g == 0:
                # anchors carry the -0.25 prescale
                iv = scr.tile([128, 4], f32, tag="iv")
                nc.vector.reciprocal(iv[:], nrme[:])
                nc.vector.tensor_scalar_mul(inv[:, s4], iv[:], -0.25)
            else:
                nc.vector.reciprocal(inv[:, s4], nrme[:])

        def group_transpose(g, evac):
            gt = xt.tile([128, 1024], f32r, tag=f"xT{g}")
            pt = tp.tile([128, 1024], f32, tag="tp")
            for i in range(4):
                t = 4 * g + i
                xn = dpool.tile([128, D], f32, tag="xn")
                if g > 0 and i >= 2:
                    nc.gpsimd.tensor_scalar_mul(
                        xn[:], xg[g][:, ts(i, D)], inv[:, t : t + 1]
                    )
                else:
                    nc.vector.tensor_scalar_mul(
                        xn[:], xg[g][:, ts(i, D)], inv[:, t : t + 1]
                    )
                for c in range(2):
                    nc.tensor.transpose(
                        pt[:, ts(c * 4 + i, 128)], xn[:, ts(c, 128)], ident[:]
                    )
            if evac == "act":
                nc.scalar.copy(gt[:], pt[:])
            elif evac == "dve":
                nc.vector.tensor_copy(gt[:], pt[:])
            else:  # split act + dve
                nc.scalar.copy(gt[:, 0:512], pt[:, 0:512])
                nc.vector.tensor_copy(gt[:, 512:1024], pt[:, 512:1024])
            xT.append(gt)

        # Groups 0..2: norms, transposes, masks.
        evac_plan = ["act", "act", "act", "act", "act"]
        for g in range(3):
            group_norms(g)
            group_transpose(g, evac_plan[g])
        for g in range(3, NG):
            group_norms(g)
            group_transpose(g, evac_plan[g])

        # --- Matmuls in halves: A = groups 1,2 (cols 0:1024), B = 3,4.
        lhs = xT[0]
        mx = stat.tile([128, RT], f32, tag="mx")
        mn = stat.tile([128, RT], f32, tag="mn")

        def half_mms(r, gi0):
            pm = pmp.tile([128, 1024], f32, tag="pm")
            for k, gi in enumerate((gi0, gi0 + 1)):
                for c in range(2):
                    nc.tensor.matmul(
                        pm[:, ts(k, 512)],
                        lhsT=lhs[:, c * 512 + r * 128 : c * 512 + (r + 1) * 128],
                        rhs=xT[gi][:, ts(c, 512)],
                        start=(c == 0),
                        stop=(c == 1),
                    )
            return pm

        pma = [half_mms(r, 1) for r in range(RT)]

        def stt(r, pm, h):
            # w = (t_o == t_b_r) + pm, fused mask with the PSUM read.
            nc.vector.scalar_tensor_tensor(
                wtiles[r][:, h * 1024 : (h + 1) * 1024],
                tob[:, h * 1024 : (h + 1) * 1024],
                tbt[:, r : r + 1],
                pm[:],
                op0=Alu.is_equal,
                op1=Alu.add,
            )

        def tree(r, op, acc):
            w = wtiles[r]
            t1 = scr.tile([128, 1024], fp16, tag="t1")
            nc.vector.tensor_tensor(t1[:], w[:, 0:1024], w[:, 1024:2048], op=op)
            t2 = scr.tile([128, 512], fp16, tag="t2")
            nc.vector.tensor_tensor(t2[:], t1[:, 0:512], t1[:, 512:1024], op=op)
            t3 = scr.tile([128, 256], fp16, tag="t3")
            nc.vector.tensor_tensor(t3[:], t2[:, 0:256], t2[:, 256:512], op=op)
            nc.vector.tensor_reduce(acc[:, r : r + 1], t3[:], axis=AxX, op=op)

        for r in range(3):
            stt(r, pma[r], 0)
        pmb = [half_mms(r, 3) for r in range(RT)]
        stt(3, pma[3], 0)
        for r in range(RT):
            stt(r, pmb[r], 1)
            tree(r, Alu.max, mx)
            tree(r, Alu.min, mn)

        # --- Epilogue on [128, RT].
        u1 = stat.tile([128, RT], f32, tag="u1")
        nc.vector.tensor_scalar(u1[:], mx[:], 8.0, -6.0, op0=Alu.mult, op1=Alu.add)
        u = stat.tile([128, RT], f32, tag="u")
        nc.vector.tensor_scalar_max(u[:], u1[:], 0.0)
        dap = stat.tile([128, RT], f32, tag="dap")
        nc.scalar.activation(dap[:], u[:], Act.Sqrt)
        v1 = stat.tile([128, RT], f32, tag="v1")
        nc.vector.tensor_scalar(v1[:], mn[:], 8.0, 2.0, op0=Alu.mult, op1=Alu.add)
        v = stat.tile([128, RT], f32, tag="v")
        nc.vector.tensor_scalar_max(v[:], v1[:], EPS)
        sv = stat.tile([128, RT], f32, tag="sv")
        nc.scalar.activation(sv[:], v[:], Act.Sqrt)
        e = stat.tile([128, RT], f32, tag="e")
        nc.vector.tensor_scalar(e[:], v[:], 6.0, None, op0=Alu.is_gt)
        ome = stat.tile([128, RT], f32, tag="ome")
        nc.vector.tensor_scalar(ome[:], e[:], -1.0, 1.0, op0=Alu.mult, op1=Alu.add)
        t1 = stat.tile([128, RT], f32, tag="t1e")
        nc.vector.tensor_tensor(t1[:], sv[:], ome[:], op=Alu.mult)
        dan = stat.tile([128, RT], f32, tag="dan")
        nc.vector.tensor_tensor(dan[:], t1[:], e[:], op=Alu.add)
        df = stat.tile([128, RT], f32, tag="df")
        nc.vector.tensor_tensor(df[:], dap[:], dan[:], op=Alu.subtract)
        lrow = stat.tile([128, RT], f32, tag="lrow")
        nc.vector.tensor_scalar(
            lrow[:], df[:], MARGIN, 0.0, op0=Alu.add, op1=Alu.max
        )
        ps2 = tp.tile([1, RT], f32, tag="tp")
        nc.tensor.matmul(ps2[:], lhsT=ones[:], rhs=lrow[:], start=True, stop=True)
        tot = stat.tile([1, 1], f32, tag="tot")
        nc.vector.tensor_reduce(tot[:], ps2[:], axis=AxX, op=Alu.add)
        nc.sync.dma_start(out[:], tot[:])

    nc.compile()
    return nc


def _get_nc():
    if "nc" not in _CACHE:
        _CACHE["nc"] = _build()
    return _CACHE["nc"]


def make_in_maps(inputs: np.ndarray, targets: np.ndarray):
    inputs = np.ascontiguousarray(inputs, dtype=np.float32)
    tf = targets.astype(np.float32)
    th = targets.astype(np.float16)
    in_maps = []
    for r in range(NCORES):
        rows = slice(r * RPC, (r + 1) * RPC)
        opp = slice(HALF, N) if r * RPC < HALF else slice(0, HALF)
        in_maps.append(
            {
                # partition p holds rows 4p..4p+3 (contiguous 4KB DMA);
                # "tile" t within a group is row 4p+t.
                "xb": inputs[rows].reshape(128, RT * D),
                "xo": inputs[opp].reshape(128, 16 * D),
                "tb": tf[rows].reshape(128, RT),
                # xo partition k holds rows 16k..16k+15; group n covers tile
                # slices 4n..4n+3, so distance column n*512 + i*128 + k is
                # xo-row 16k + 4n + i: permute targets to match.
                "to": th[opp].reshape(128, 4, 4).transpose(1, 2, 0).reshape(1, HALF),
            }
        )
    return in_maps


def kernel(inputs: np.ndarray, targets: np.ndarray) -> np.ndarray:
    from concourse.bass_utils import run_bass_kernel_spmd

    nc = _get_nc()
    in_maps = make_in_maps(inputs, targets)
    res = run_bass_kernel_spmd(nc, in_maps, list(range(NCORES)))
    total = sum(float(res.results[i]["out"][0, 0]) for i in range(NCORES))
    return np.float32(total / N)
